# revision 13
# baseline (speedup 1.0000x reference)
"""Trainium2 Bass kernel for the Gaussian-mixture ray autoencoder.
Sparse (ray-tiled, certified-pruned) version.

Math: prob[n] = sigmoid( sum_k lab_k * exp(-0.5 (pos_n-mu_k)^T Sigma_k^{-1} (pos_n-mu_k)) )

Key idea: the Cholesky factors are tiny (sigma ~ 0.01..0.2), so exp(-q/2)
is negligible for ~75-97%% of (ray, gaussian) pairs.  Rays are clustered
into 64 spatial tiles of 128 (balanced kd-split on the 4D (origin,dir)
point); for each tile we keep only the gaussians whose CERTIFIED minimum
Mahalanobis distance over the tile's bounding boxes (4 sub-boxes of 32
rays, min of projected-gradient box-QP lower bounds) is small enough.
The dropped per-ray mass is rigorously bounded by sum_k exp(-qbound/2)
<= BUDGET, chosen per tile (adaptive threshold).

Device schedule per core (8 slots = 8 tiles of 128 rays):
 - PE: per (slot, band g in 0..3) ONE native-fp32 C=16 matmul (the PE
   decomposes fp32 into HIGH/LOW passes internally at full precision)
   builds the slot's [128, Wj] q block in PSUM (4 row-group-concurrent
   bands; Wj = 2*W'_j, pos block then neg block each padded to W'_j
   with dummy exp->0 columns).
 - ScalarE: ONE Exp per slot straight from PSUM into fp16 scratch.
 - VectorE: one reduce_sum per slot with o=2 groups -> (pos_sum, neg_sum).
 - Host: S = pos-neg (+ exact f64 correction for capacity overflow),
   sigmoid, un-permute rays.
"""

import os
import sys

import numpy as np

if "/opt/trn_rl_repo" not in sys.path:
    sys.path.insert(0, "/opt/trn_rl_repo")

N = 8192
K = 4096
NCORES = 8
NSLOT = 8                    # ray tiles per core
NTILES = NCORES * NSLOT      # 64
TILE = N // NTILES           # 128 rays per tile
SUBLEAF = int(os.environ.get("KERNEL_SUBLEAF", "16"))
BUDGET = float(os.environ.get("KERNEL_BUDGET", "1e-2"))
WCAP = 1016                  # per-sign per-slot column cap (PSUM half = 2048)
PGD_ITERS = int(os.environ.get("KERNEL_PGD_ITERS", "25"))
N_WARMUP = int(os.environ.get("KERNEL_WARMUP", "1"))
FP32_PROBE = os.environ.get("KERNEL_FP32", "0") == "1"
WAIT_OSEM = os.environ.get("KERNEL_WAIT_OSEM", "0") == "1"

# index pairs for the quadratic monomials p_i * p_j
_IU = [(0, 0), (1, 1), (2, 2), (3, 3),
       (0, 1), (0, 2), (0, 3), (1, 2), (1, 3), (2, 3)]

# slot j gets the tile-width rank RANK_OF_SLOT[j] (0 = widest group)
RANK_OF_SLOT = [6, 4, 2, 0, 1, 3, 5, 7]

LAST_EXEC_TIME_NS = None
_GRAPH_CACHE = {}


def _round_f32r(x):
    from neuronxcc.starfish.support.dtype import (
        static_cast_fp32_to_fp32r,
        static_cast_fp32r_to_fp32,
    )

    x32 = np.ascontiguousarray(x, dtype=np.float32)
    return np.asarray(
        static_cast_fp32r_to_fp32(static_cast_fp32_to_fp32r(x32)), dtype=np.float32
    )


def _kd_leaves(pos, target):
    """Balanced kd split into leaves of exactly `target` points (N is a
    power-of-2 multiple of target). Returns list of index arrays, in
    spatial traversal order."""
    leaves = []

    def split(ids):
        if len(ids) <= target:
            leaves.append(ids)
            return
        P = pos[ids]
        dim = int(np.argmax(P.max(0) - P.min(0)))
        order = np.argsort(P[:, dim], kind="stable")
        half = len(ids) // 2
        split(ids[order[:half]])
        split(ids[order[half:]])

    split(np.arange(len(pos)))
    return leaves


def _certified_bounds(pos, mu, A, leaves):
    """For each (leaf-box, gaussian): a certified lower bound on
    min_{p in box} (p-mu)^T A (p-mu), via projected gradient descent +
    the convexity (first-order) bound at the final iterate."""
    S = len(leaves)
    los = np.stack([pos[ids].min(0) for ids in leaves]).astype(np.float32)
    his = np.stack([pos[ids].max(0) for ids in leaves]).astype(np.float32)
    A32 = A.astype(np.float32)
    mu32 = mu.astype(np.float32)
    lo = los[:, None, :]
    hi = his[:, None, :]
    lam_max = np.linalg.eigvalsh(A32)[:, -1]
    step = (1.0 / (2.0 * lam_max))[None, :, None].astype(np.float32)
    p = np.clip(mu32[None, :, :], lo, hi)
    for _ in range(PGD_ITERS):
        g = 2.0 * np.einsum("kde,ske->skd", A32, p - mu32[None, :, :])
        p = np.clip(p - step * g, lo, hi)
    d = p - mu32[None, :, :]
    g = 2.0 * np.einsum("kde,ske->skd", A32, d)
    qp = np.einsum("skd,skd->sk", d, 0.5 * g)
    slack = np.minimum(g * (lo - p), g * (hi - p)).sum(-1)
    # 0.5 safety margin for fp32 arithmetic slop in the bound itself
    return np.maximum(qp + slack - 0.5, 0.0)


def _host_prep(origins, directions, embeddings, chol, labels, idx):
    idx = np.asarray(idx).astype(np.int64)
    mu = np.asarray(embeddings, dtype=np.float64)[idx]        # [K,4]
    L = np.asarray(chol, dtype=np.float64)[idx]               # [K,4,4]
    lab = np.asarray(labels, dtype=np.float64)[idx]           # [K]

    Sigma = np.einsum("kij,klj->kil", L, L)
    A = np.linalg.inv(Sigma)                                  # [K,4,4]

    pos = np.concatenate(
        [np.asarray(origins, np.float64), np.asarray(directions, np.float64)], axis=1
    )                                                         # [N,4]
    center = 0.5
    pos_c = pos - center
    mu_c = mu - center

    b = np.einsum("kij,kj->ki", A, mu_c)                      # [K,4]
    c = np.einsum("ki,ki->k", mu_c, b)                        # [K]

    kk = idx.shape[0]
    W = np.zeros((16, kk), dtype=np.float64)
    for r, (i, j) in enumerate(_IU):
        W[r] = -0.5 * A[:, i, j] if i == j else -A[:, i, j]
    W[10:14] = b.T
    with np.errstate(divide="ignore"):
        absl = np.abs(np.where(lab == 0, 1.0, lab))
        loglab = np.where(lab == 0.0, -1e5, np.log(absl))
    W[14] = -0.5 * c + loglab

    F = np.zeros((16, N), dtype=np.float64)
    for r, (i, j) in enumerate(_IU):
        F[r] = pos_c[:, i] * pos_c[:, j]
    F[10:14] = pos_c.T
    F[14] = 1.0

    sgn = np.sign(lab)

    # --- spatial tiling + certified pruning ---
    leaves = _kd_leaves(pos, SUBLEAF)                          # 256 x 32
    per_tile = TILE // SUBLEAF                                 # 4 sub-leaves/tile
    qb_sub = _certified_bounds(pos, mu, A, leaves)             # [256, K]
    qbound = qb_sub.reshape(NTILES, per_tile, kk).min(1)       # [64, K]

    perm = np.concatenate(leaves)                              # ray permutation

    keep_pos = []   # per tile: kept pos gaussian ids (by qbound asc)
    keep_neg = []
    over_ids = []   # per tile: capacity-overflow ids (host-corrected)
    for t in range(NTILES):
        qb = qbound[t]
        order = np.argsort(qb, kind="stable")
        mass = np.exp(-0.5 * qb[order])
        suffix = np.cumsum(mass[::-1])[::-1]
        m = int(np.searchsorted(-suffix, -BUDGET))
        kept = order[:m]
        kp = kept[sgn[kept] > 0]
        kn = kept[sgn[kept] < 0]
        ov = []
        if len(kp) > WCAP:
            ov.append(kp[WCAP:])
            kp = kp[:WCAP]
        if len(kn) > WCAP:
            ov.append(kn[WCAP:])
            kn = kn[:WCAP]
        keep_pos.append(kp)
        keep_neg.append(kn)
        over_ids.append(np.concatenate(ov) if ov else np.empty(0, np.int64))

    # --- slot assignment: rank tiles by width, group ranks of 8; slot
    # order small-first (fast fill), biggest mid, smallest last (short
    # tail): slot j holds rank RANK_OF_SLOT[j] ---
    wmax = np.array([max(len(keep_pos[t]), len(keep_neg[t]), 1)
                     for t in range(NTILES)])
    order_t = np.argsort(-wmax, kind="stable")
    tile_of = np.empty((NCORES, NSLOT), dtype=np.int64)
    Wq = []
    for j in range(NSLOT):
        rank = RANK_OF_SLOT[j]
        grp = order_t[8 * rank: 8 * rank + 8]
        for ci, t in enumerate(grp):
            tile_of[ci, j] = t
        w = int(wmax[grp].max())
        w = min(-(-w // 4) * 4, WCAP)     # mult of 4 -> band chunks even
        Wq.append(max(w, 4))
    Wq = tuple(Wq)

    # --- host correction for overflow (exact f64, normally empty) ---
    S_extra = np.zeros(N, dtype=np.float64)
    for t in range(NTILES):
        ids = over_ids[t]
        if len(ids):
            rays = perm[t * TILE:(t + 1) * TILE]
            q = F[:, rays].T @ W[:, ids]                       # [128, nov]
            S_extra[rays] += (sgn[ids][None, :] * np.exp(q)).sum(1)

    return dict(W=W, F=F, sgn=sgn, perm=perm, tile_of=tile_of, Wq=Wq,
                keep_pos=keep_pos, keep_neg=keep_neg, S_extra=S_extra)


# --- device graph -----------------------------------------------------------
# wf column layout per slot j: [F1_j (128) | W_j (wb_j) | F2_j (128)],
# slot blocks sequential.  W_j: band g in rows [32g,32g+16)=Whi,
# [+16,+32)=Wlo, all bands in the same wb_j columns.  F1: Fhi duplicated
# in hi and lo 16-row halves; F2: Flo in hi halves only.

def _layout(Wq):
    f1c, wc = [], []
    cur = 1                                # col 0: exp bias (zeros)
    wb = [w * 2 // 4 for w in Wq]          # slot width 2W' split over 4 bands
    for j in range(NSLOT):
        f1c.append(cur); cur += 128
        wc.append(cur); cur += wb[j]
    return f1c, wc, wb, cur


def _chunks(wb):
    """ACT exp chunk table: (slot, first_bank, n_banks). Slot 0 is split
    so its first bank's exp can start before the other row-band DMA
    pieces land."""
    ch = [(0, 0, 1), (0, 1, 3)]
    for j in range(1, NSLOT - 1):
        ch.append((j, 0, 4))
    ch.append((NSLOT - 1, 0, 2))
    ch.append((NSLOT - 1, 2, 2))
    cum = {}
    n = 0
    for (j, b0, nb) in ch:
        n += 1
        cum[j] = n
    return ch, cum


def _build_graph_raw(Wq):
    import concourse.bass as bass
    import concourse.mybir as mybir

    f32 = mybir.dt.float32
    f32r = mybir.dt.float32
    f16 = mybir.dt.float16
    Exp = mybir.ActivationFunctionType.Exp

    f1c, wc, wb, X = _layout(Wq)
    Wj = [4 * b for b in wb]               # PSUM width per slot (= 2*W'_j)
    poff = [2048 * (j % 2) for j in range(NSLOT)]
    soff = np.concatenate([[0], np.cumsum(Wj)]).astype(int)
    SCR = int(soff[-1])

    chunks, cum = _chunks(wb)

    nc = bass.Bass()
    wfd = nc.declare_dram_parameter("wf", [128, X], f32r, isOutput=False)
    outd = nc.declare_dram_parameter("out", [128, 2 * NSLOT], f32, isOutput=True)

    from contextlib import ExitStack

    with ExitStack() as stack:
        wfsb = stack.enter_context(nc.sbuf_tensor("wfsb", [128, X], f32r))
        scratch = stack.enter_context(nc.sbuf_tensor("scratch", [128, SCR], f16))
        sums = stack.enter_context(nc.sbuf_tensor("sums", [128, 2 * NSLOT], f32))
        dummy = stack.enter_context(nc.sbuf_tensor("warm_act", [128, 1], f32))
        psall = stack.enter_context(nc.psum_tensor("psall", [128, 4096], f32))
        sem_names = (
            ["dbias", "dR0", "dR1", "dR2", "dR3"]
            + [f"ds{i}" for i in range(1, 8)]
            + ["psem", "asem", "rsem", "osem"]
        )
        sems = {s: stack.enter_context(nc.semaphore(s)) for s in sem_names}
        dbias = sems["dbias"]
        psem, asem, rsem, osem = (
            sems["psem"], sems["asem"], sems["rsem"], sems["osem"]
        )
        dR = [sems[f"dR{g}"] for g in range(4)]
        ds = [None] + [sems[f"ds{i}"] for i in range(1, 8)]
        block = stack.enter_context(nc.Block(no_gpsimd_drain=True))
        s0_end = wc[0] + wb[0]           # slot-0 block = cols [1, s0_end)

        def sblk(j):
            return (f1c[j], wc[j] + wb[j])

        @block.scalar
        def _(scalar):
            # bias col + critical slice (rows 0:32 of slot-0 block), then
            # the Exp table load/warm; no const memsets anywhere
            scalar.dma_start(
                out=wfsb[0:16, 1:s0_end], in_=wfd[0:16, 1:s0_end]
            ).then_inc(dR[0], 16)
            scalar.dma_start(
                out=wfsb[32:48, 1:s0_end], in_=wfd[32:48, 1:s0_end]
            ).then_inc(dR[1], 16)
            scalar.dma_start(
                out=wfsb[96:112, 1:s0_end], in_=wfd[96:112, 1:s0_end]
            ).then_inc(dR[3], 16)
            scalar.activation(dummy[:], dummy[:], Exp, scale=0.0)
            a6, b6 = sblk(6)
            scalar.dma_start(
                out=wfsb[0:112, a6:b6], in_=wfd[0:112, a6:b6]
            ).then_inc(ds[6], 16)
            nhalf = 0
            for ci, (j, b0, nb) in enumerate(chunks):
                scalar.wait_ge(psem, 4 * j + b0 + nb)
                # band chunks are bank-strided in PSUM; compact them into
                # contiguous scratch via matching 3D APs
                src = psall[
                    :, poff[j] + 512 * b0: poff[j] + 512 * (b0 + nb)
                ].rearrange("p (o f) -> p o f", o=nb)[:, :, 0: wb[j]]
                dst = scratch[
                    :,
                    int(soff[j]) + b0 * wb[j]: int(soff[j]) + (b0 + nb) * wb[j],
                ].rearrange("p (o f) -> p o f", o=nb)
                acc = None
                if j == NSLOT - 1:
                    # last slot: pos/neg halves summed on the ACT
                    # accumulator, no DVE reduce needed
                    acc = sums[:, 2 * j + nhalf: 2 * j + nhalf + 1]
                    nhalf += 1
                scalar.activation(dst, src, Exp, accum_out=acc).then_inc(asem)
            scalar.sem_clear(psem)

        @block.gpsimd
        def _(gpsimd):
            a7, b7 = sblk(7)
            gpsimd.dma_start(
                out=wfsb[0:112, a7:b7], in_=wfd[0:112, a7:b7]
            ).then_inc(ds[7], 16)

        @block.sync
        def _(sync):
            sync.dma_start(
                out=wfsb[64:80, 1:s0_end], in_=wfd[64:80, 1:s0_end]
            ).then_inc(dR[2], 16)
            for j in (1, 2, 3, 4, 5):
                a, b = sblk(j)
                sync.dma_start(
                    out=wfsb[0:112, a:b], in_=wfd[0:112, a:b]
                ).then_inc(ds[j], 16)
            sync.wait_ge(rsem, NSLOT - 2)
            sync.dma_start(out=outd[:, 0: 2 * NSLOT - 4],
                           in_=sums[:, 0: 2 * NSLOT - 4]).then_inc(osem, 16)
            sync.wait_ge(rsem, NSLOT - 1)
            sync.wait_ge(asem, len(chunks))
            sync.sem_clear(rsem)
            sync.sem_clear(asem)
            sync.dma_start(out=outd[:, 2 * NSLOT - 4: 2 * NSLOT],
                           in_=sums[:, 2 * NSLOT - 4: 2 * NSLOT]).then_inc(osem, 16)
            if WAIT_OSEM:
                sync.wait_ge(osem, 16)
                sync.sem_clear(osem)

        @block.vector
        def _(vector):
            for j in range(NSLOT - 1):
                vector.wait_ge(asem, cum[j])
                src = scratch[:, int(soff[j]): int(soff[j]) + Wj[j]]
                vector.reduce_sum(
                    sums[:, 2 * j: 2 * j + 2],
                    src.rearrange("p (o f) -> p o f", o=2),
                    axis=mybir.AxisListType.X,
                ).then_inc(rsem)

        @block.tensor
        def _(tensor):
            waited = set()

            def wait_once(sem, val, key):
                if key not in waited:
                    tensor.wait_ge(sem, val)
                    tensor.sem_clear(sem)
                    waited.add(key)

            wwarm = min(wb[0], 512)
            for i in range(3 * N_WARMUP):
                g = 1 + i % 3
                tensor.matmul(
                    psall[:, 2048 + 512 * (g - 1): 2048 + 512 * (g - 1) + wwarm],
                    lhsT=wfsb[32 * g: 32 * g + 16, f1c[0]: f1c[0] + 128],
                    rhs=wfsb[32 * g: 32 * g + 16, wc[0]: wc[0] + wwarm],
                    start=True, stop=True,
                    tile_position=(32 * g, 0),
                )

            def amm(j, g):
                # one native-fp32 C=16 pass per band; dst stays inside a
                # single PSUM bank (fp32 matmul restriction)
                tensor.matmul(
                    psall[:, poff[j] + 512 * g: poff[j] + 512 * g + wb[j]],
                    lhsT=wfsb[32 * g: 32 * g + 16, f1c[j]: f1c[j] + 128],
                    rhs=wfsb[32 * g: 32 * g + 16, wc[j]: wc[j] + wb[j]],
                    start=True, stop=True, tile_position=(32 * g, 0),
                ).then_inc(psem)

            for j in range(NSLOT):
                if j == 0:
                    # band 0 complete first so the head exp chunk starts
                    # as soon as its row-piece lands
                    for g in range(4):
                        wait_once(dR[g], 16, f"R{g}")
                        amm(0, g)
                    continue
                wait_once(ds[j], 16, f"s{j}")
                if j >= 2:
                    tensor.wait_ge(asem, cum[j - 2])
                for g in range(4):
                    amm(j, g)

    _strip_exit_barrier(nc, __import__("concourse.mybir", fromlist=["x"]))
    _legalize_waits(nc, __import__("concourse.mybir", fromlist=["x"]))
    return nc


def _strip_exit_barrier(nc, mybir):
    def is_exit_inst(i, in_end_bb):
        if isinstance(i, mybir.InstDrain):
            return True
        if isinstance(i, mybir.InstEventSemaphore):
            if in_end_bb:
                return True
            si = i.sync_info
            for grp in ((si.on_wait if si else []) or []), ((si.on_update if si else []) or []):
                for w in grp:
                    nm = getattr(w, "ant_name", "") or ""
                    if "barrier_" in nm:
                        return True
        return False

    for fn in nc.m.functions:
        for bb in fn.blocks:
            end = bb.name.endswith("_end")
            bb.instructions = [
                i for i in bb.instructions if not is_exit_inst(i, end)
            ]


def _legalize_waits(nc, mybir):
    cnt = [0]
    for fn in nc.m.functions:
        for bb in fn.blocks:
            new = []
            for ins in bb.instructions:
                si = ins.sync_info
                if si is not None and si.on_wait and len(si.on_wait) > 1:
                    waits = list(si.on_wait)
                    for w in waits[:-1]:
                        cnt[0] += 1
                        nop = mybir.InstNoOp(
                            name=f"I-waitfix-{cnt[0]}",
                            engine=ins.engine,
                            sync_info=mybir.SyncInfo(on_wait=[w], on_update=[]),
                        )
                        new.append(nop)
                    si.on_wait = [waits[-1]]
                new.append(ins)
            bb.instructions = new


def _ensure_ntff_hook():
    try:
        from antenv.axon_hooks import get_axon_ntff_profile_hook  # noqa: F401
        return
    except ImportError:
        pass
    import types

    import antenv

    mod = types.ModuleType("antenv.axon_hooks")
    mod._hook = None

    def set_axon_ntff_profile_hook(h):
        mod._hook = h

    def get_axon_ntff_profile_hook():
        return mod._hook

    mod.set_axon_ntff_profile_hook = set_axon_ntff_profile_hook
    mod.get_axon_ntff_profile_hook = get_axon_ntff_profile_hook
    sys.modules["antenv.axon_hooks"] = mod
    antenv.axon_hooks = mod
    try:
        from trn_agent_boot.trn_boot import _ntff_profile_via_ctypes

        hook = _ntff_profile_via_ctypes("/opt/axon/libaxon_pjrt.so")
        if hook is not None:
            mod._hook = hook
    except Exception:
        pass


def _make_in_maps(prep):
    W, F = prep["W"], prep["F"]
    Wq, tile_of, perm = prep["Wq"], prep["tile_of"], prep["perm"]
    keep_pos, keep_neg = prep["keep_pos"], prep["keep_neg"]

    f1c, wc, wb, X = _layout(Wq)

    Whi_all = W.astype(np.float32)
    Fhi = F.astype(np.float32)

    in_maps = []
    for c in range(NCORES):
        buf = np.zeros((128, X), dtype=np.float32)
        for j in range(NSLOT):
            t = int(tile_of[c, j])
            rays = perm[t * TILE:(t + 1) * TILE]
            Wp = Wq[j]
            wjj = 2 * Wp
            # padded slot W matrix [16, wjj]
            Whi = np.zeros((16, wjj), dtype=np.float32)
            Whi[14, :] = -60000.0              # dummy cols: exp -> 0
            kp, kn = keep_pos[t], keep_neg[t]
            Whi[:, :len(kp)] = Whi_all[:, kp]
            Whi[:, Wp:Wp + len(kn)] = Whi_all[:, kn]
            for g in range(4):
                hi = slice(32 * g, 32 * g + 16)
                cw = wc[j]
                bs = slice(g * wb[j], (g + 1) * wb[j])
                buf[hi, cw: cw + wb[j]] = Whi[:, bs]
                buf[hi, f1c[j]: f1c[j] + 128] = Fhi[:, rays]
        in_maps.append({"wf": buf})
    return in_maps


def kernel(origins, directions, embeddings, chol, labels, idx):
    global LAST_EXEC_TIME_NS
    import concourse.bass_utils as bass_utils
    from concourse.bass_utils import run_bass_kernel_spmd

    prep = _host_prep(origins, directions, embeddings, chol, labels, idx)
    Wq = prep["Wq"]

    if Wq not in _GRAPH_CACHE:
        _GRAPH_CACHE[Wq] = _build_graph_raw(Wq)
    nc = _GRAPH_CACHE[Wq]

    in_maps = _make_in_maps(prep)

    trace = os.environ.get("KERNEL_TRACE", "0") == "1"
    if trace:
        _ensure_ntff_hook()
        bass_utils.upload_artifacts = lambda tmpdir: tmpdir
    res = run_bass_kernel_spmd(nc, in_maps, core_ids=list(range(NCORES)), trace=trace)
    LAST_EXEC_TIME_NS = res.exec_time_ns

    perm, tile_of = prep["perm"], prep["tile_of"]
    S_extra = prep["S_extra"]
    out = np.empty((N,), dtype=np.float32)
    for c in range(NCORES):
        oc = np.asarray(res.results[c]["out"], dtype=np.float64)  # [128, 16]
        for j in range(NSLOT):
            t = int(tile_of[c, j])
            rays = perm[t * TILE:(t + 1) * TILE]
            S = oc[:, 2 * j] - oc[:, 2 * j + 1] + S_extra[rays]
            out[rays] = (1.0 / (1.0 + np.exp(-S))).astype(np.float32)
    return out.reshape(-1, 1)
